# revision 102
# baseline (speedup 1.0000x reference)
"""Multi-head causal self-attention on 8 TRN2 NeuronCores — fp8 DoubleRow.

Problem (nn_MultiHeadAttention): B=2, T=2048, C=1024, H=16 heads, hs=64.
  q,k,v = per-head projections of x; causal softmax(q k^T / 8) v;
  concat heads; out = att @ Wo + bo.

Sharding: core c in 0..7 -> (batch b = c//4, head-group g = c%4, 4 heads).
Per core: flash-style causal attention for its 4 heads, AllGather of the
normalized attention outputs across the 4 cores of the same batch, then a
disjoint 256-column slice of the output projection. Host concats slices.

Numerics (measured end-to-end rel-err 9.4e-3 vs 2e-2 budget):
  host:  x_hi=fp8(16x), x_lo=fp8(16x-x_hi); w{q,k,v}_hi=fp8(1024 W),
         w_lo=fp8(1024W - w_hi).  fp8 = e4m3; scales keep values in the
         e4m3 normal range (w~0.02 would otherwise land subnormal).
  QKV projections: fully error-compensated fp8 DoubleRow matmuls
         (w_hi.x_hi + w_hi.x_lo + w_lo.x_hi: 3 slot-products per K=128,
         12 DoubleRow instrs per 512-wide tile vs 8 f32r = 0.75x cycles,
         and each DoubleRow instr costs out_free/2 cycles = overall 2.67x
         fewer PE cycles than f32r).
  scores: q requantized to fp8 (q-side error only), k split hi/lo on
         device; DoubleRow lhsT=(k_hi|k_lo), rhs=(q|q dup) — 2x fewer
         cycles, diagonal tiles column-sliced to the causal region.
  P=exp(S/8): ACT, bf16 out, both head-pairs in one instruction.
  AV, output projection: bf16 (1:1 error transfer paths stay >=bf16).
  normalize: DVE reciprocal -> Pool partition_broadcast -> DVE multiply
         (no PE broadcast matmul, no PSUM->SBUF staging copy).

Scheduling: x/w are SBUF-resident (loaded once, ~46KB/partition), so
stage-1 (QKV) and stage-3 (out-proj) chunks are woven into the
scores->exp->AV s-loops to fill PE bubbles, as in the f32r baseline.
"""

import numpy as np
import ml_dtypes
from contextlib import ExitStack

import concourse.bass as bass
import concourse.mybir as mybir
import concourse.tile as tile
from concourse import bacc
from concourse.bass_utils import run_bass_kernel_spmd

F32 = mybir.dt.float32
F32R = mybir.dt.float32r
BF16 = mybir.dt.bfloat16
FP8 = mybir.dt.float8e4
EXP = mybir.ActivationFunctionType.Exp
DR = mybir.MatmulPerfMode.DoubleRow
E4 = ml_dtypes.float8_e4m3
BF = ml_dtypes.bfloat16

N_CORES = 8
B = 2
T = 2048
C = 1024
NH = 16
HS = 64
E = 1024
GROUPS = 4          # head groups (tensor-parallel ranks per batch)
HPG = NH // GROUPS  # 4 heads per core
ES = E // GROUPS    # 256 output columns per core
HD = HPG * HS       # 256 local attention-output rows

P = 128             # partition tile
TBLK = 512          # t-block (matmul moving dim)
NTB = T // TBLK     # 4
NCT = C // P        # 8 contraction tiles for projections
NST = T // P        # 16 key tiles
VW = HS + 1         # V lhsT width per head (64 V cols + ones col)

SX = 16.0           # x fp8 scale
SW = 1024.0         # weight fp8 scale
QK_CAST = 1.0 / 1024.0      # psum (x*w = 2^14 q) -> fp8 storage at 16 q
EXP_SCALE = 0.125 / 256.0   # scores psum = 256 * S_raw
V_CAST = 1.0 / 16384.0      # v psum -> natural-scale bf16

REPLICA_GROUPS = [[0, 1, 2, 3], [4, 5, 6, 7]]


def build_nc(with_collective=True):
    """Build + compile the per-core SPMD program. Same program on all cores."""
    nc = bacc.Bacc(
        "TRN2", target_bir_lowering=False, debug=False, num_devices=N_CORES
    )

    # x8: rows c=(ci,p), cols (hl, tb, t) — hi/lo-major so DoubleRow slot
    # pairs (hi,lo) and (ci,ci+1) are both expressible as free-dim strides
    x8 = nc.dram_tensor("x8", [C, 2 * T], FP8, kind="ExternalInput").ap()
    # w hi duplicated per ci (DoubleRow slots need physical duplication)
    wqh = nc.dram_tensor("wqh", [C, 2 * HD], FP8, kind="ExternalInput").ap()
    wkh = nc.dram_tensor("wkh", [C, 2 * HD], FP8, kind="ExternalInput").ap()
    wvh = nc.dram_tensor("wvh", [C, 2 * HD], FP8, kind="ExternalInput").ap()
    wql = nc.dram_tensor("wql", [C, HD], FP8, kind="ExternalInput").ap()
    wkl = nc.dram_tensor("wkl", [C, HD], FP8, kind="ExternalInput").ap()
    wvl = nc.dram_tensor("wvl", [C, HD], FP8, kind="ExternalInput").ap()
    wo = nc.dram_tensor("wo", [E, ES], BF16, kind="ExternalInput").ap()
    bo = nc.dram_tensor("bo", [1, ES], BF16, kind="ExternalInput").ap()
    tri2 = nc.dram_tensor("tri2", [P, 2 * P], BF16, kind="ExternalInput").ap()
    out = nc.dram_tensor("out", [T, ES], F32, kind="ExternalOutput").ap()

    with tile.TileContext(nc) as tc, ExitStack() as ctx:
        wp = ctx.enter_context(tc.tile_pool(name="wp", bufs=1))
        qkp = ctx.enter_context(tc.tile_pool(name="qkp", bufs=1))
        vp = ctx.enter_context(tc.tile_pool(name="vp", bufs=1))
        ptp = ctx.enter_context(tc.tile_pool(name="ptp", bufs=10))
        attp = ctx.enter_context(tc.tile_pool(name="attp", bufs=4))
        smp = ctx.enter_context(tc.tile_pool(name="smp", bufs=4))
        outp = ctx.enter_context(tc.tile_pool(name="outp", bufs=4))
        lhp = ctx.enter_context(tc.tile_pool(name="lhp", bufs=16))
        # PSUM: 8 banks. st2 [128,1024] = 2 banks x 2 bufs = 4,
        # attv 1 bank x 2, small (qkv proj / out-proj) 1 bank x 2.
        ps2 = ctx.enter_context(tc.tile_pool(name="ps2", bufs=2, space="PSUM"))
        psB = ctx.enter_context(tc.tile_pool(name="psB", bufs=2, space="PSUM"))
        psC = ctx.enter_context(tc.tile_pool(name="psC", bufs=2, space="PSUM"))
        dramp = ctx.enter_context(tc.tile_pool(name="dramp", bufs=1,
                                               space="DRAM"))

        # ---- SBUF-resident inputs ----
        x_sb = wp.tile([P, 2 * NCT * NTB * TBLK], FP8, tag="x")

        def x_ap():  # [p, hl, ci, tb, t]
            return x_sb[:].rearrange(
                "p (hl ci tb t) -> p hl ci tb t", hl=2, ci=NCT, tb=NTB)

        wqh_sb = wp.tile([P, NCT * 2 * HD], FP8, tag="wqh")
        wkh_sb = wp.tile([P, NCT * 2 * HD], FP8, tag="wkh")
        wvh_sb = wp.tile([P, NCT * 2 * HD], FP8, tag="wvh")
        wql_sb = wp.tile([P, NCT * HD], FP8, tag="wql")
        wkl_sb = wp.tile([P, NCT * HD], FP8, tag="wkl")
        wvl_sb = wp.tile([P, NCT * HD], FP8, tag="wvl")
        wo_sb = wp.tile([P, NCT * ES], BF16, tag="wo")
        bias_sb = wp.tile([1, ES], BF16, tag="bias")
        bias_bc = wp.tile([P, ES], BF16, tag="bias_bc")
        tri_sb = wp.tile([P, 2 * P], BF16, tag="tri")
        ones = wp.tile([1, P], BF16, tag="ones")

        def whi_ap(t):  # [p, ci, pr, two, m] (m=128 = pair cols)
            return t[:].rearrange(
                "p (ci pr two m) -> p ci pr two m", ci=NCT, pr=2, two=2)

        def wlo_ap(t):  # [p, ci, pr, m]
            return t[:].rearrange("p (ci pr m) -> p ci pr m", ci=NCT, pr=2)

        def wvh_ap():  # [p, ci, two, n] (n=256)
            return wvh_sb[:].rearrange(
                "p (ci two n) -> p ci two n", ci=NCT, two=2)

        def wvl_ap():  # [p, ci, n]
            return wvl_sb[:].rearrange("p (ci n) -> p ci n", ci=NCT)

        def wo_ap():
            return wo_sb[:].rearrange("p (ci n) -> p ci n", ci=NCT)

        # q fp8, duplicated for DoubleRow rhs slots: [p(2 heads), tb, 2, t]
        q8 = [qkp.tile([P, NTB * 2 * TBLK], FP8, tag=f"q8_{pr}",
                       name=f"q8_{pr}") for pr in range(2)]
        # k hi|lo per s-tile: [p(2 heads), st, 2, s(128)]
        k8 = [qkp.tile([P, NST * 2 * P], FP8, tag=f"k8_{pr}",
                       name=f"k8_{pr}") for pr in range(2)]

        def q8_ap(pr):
            return q8[pr][:].rearrange(
                "p (tb two t) -> p tb two t", tb=NTB, two=2)

        def k8_ap(pr):
            return k8[pr][:].rearrange(
                "p (st two s) -> p st two s", st=NST, two=2)

        # v (+ ones col) bf16: [p(s), st, h, VW]
        v_sb = vp.tile([P, NST * HPG * VW], BF16, tag="v")

        def v_ap():
            return v_sb[:].rearrange(
                "p (st h w) -> p st h w", st=NST, h=HPG)

        # ---------------- stage-1 pieces ----------------
        def emit_qk_proj(tb, pr, which):
            """q^T or k^T for head pair pr of t-block tb: [128, 512] PSUM
            via 12 fully-compensated fp8 DoubleRow matmuls, then requantize
            to fp8 (q duplicated by a Pool copy; k split hi/lo)."""
            wh_sb, wl_sb = ((wqh_sb, wql_sb), (wkh_sb, wkl_sb))[which]
            wh, wl = whi_ap(wh_sb), wlo_ap(wl_sb)
            xa = x_ap()
            ps = psC.tile([P, TBLK], F32, tag="small",
                          name=f"qkps{tb}_{pr}_{which}")
            n_in = 3 * (NCT // 2)
            i = 0
            for cp in range(NCT // 2):
                c0, c1 = 2 * cp, 2 * cp + 1
                for lhsT, rhs in (
                    (wh[:, c0, pr], xa[:, :, c0, tb]),          # w_hi.(x_hi+x_lo) c0
                    (wl[:, c0:c1 + 1, pr], xa[:, 0, c0:c1 + 1, tb]),  # w_lo.x_hi
                    (wh[:, c1, pr], xa[:, :, c1, tb]),          # w_hi.(x_hi+x_lo) c1
                ):
                    nc.tensor.matmul(
                        ps[:], lhsT=lhsT, rhs=rhs,
                        start=(i == 0), stop=(i == n_in - 1), perf_mode=DR,
                    )
                    i += 1
            with nc.allow_low_precision(reason="fp8 requantization of q/k "
                                        "is the measured-error design"):
                if which == 0:
                    nc.vector.tensor_scalar_mul(
                        q8_ap(pr)[:, tb, 0], ps[:], QK_CAST)
                    nc.gpsimd.tensor_copy(
                        q8_ap(pr)[:, tb, 1], q8_ap(pr)[:, tb, 0])
                else:
                    ka = k8_ap(pr)[:, 4 * tb:4 * tb + 4]  # [p, 4, 2, 128]
                    psv = ps[:].rearrange("p (st s) -> p st s", st=4)
                    nc.vector.tensor_scalar_mul(ka[:, :, 0], psv, QK_CAST)
                    nc.vector.scalar_tensor_tensor(
                        ka[:, :, 1], psv, QK_CAST, ka[:, :, 0],
                        op0=mybir.AluOpType.mult,
                        op1=mybir.AluOpType.subtract,
                    )

        def emit_v_proj(st):
            """v^T for s-tile st: [128(t), 256] PSUM via 12 compensated
            DoubleRow matmuls, cast to natural-scale bf16 into v_sb."""
            tb, sl = st // 4, (st % 4) * P
            xa = x_ap()
            wh, wl = wvh_ap(), wvl_ap()
            vps = psC.tile([P, HD], F32, tag="small", name=f"vps{st}")
            n_in = 3 * (NCT // 2)
            i = 0
            for cp in range(NCT // 2):
                c0, c1 = 2 * cp, 2 * cp + 1
                for lhsT, rhs in (
                    (xa[:, :, c0, tb, sl:sl + P], wh[:, c0]),
                    (xa[:, 0, c0:c1 + 1, tb, sl:sl + P], wl[:, c0:c1 + 1]),
                    (xa[:, :, c1, tb, sl:sl + P], wh[:, c1]),
                ):
                    nc.tensor.matmul(
                        vps[:], lhsT=lhsT, rhs=rhs,
                        start=(i == 0), stop=(i == n_in - 1), perf_mode=DR,
                    )
                    i += 1
            with nc.allow_low_precision(reason="bf16 V is the measured-"
                                        "error design"):
                nc.vector.tensor_scalar_mul(
                    v_ap()[:, st, :, 0:HS],
                    vps[:].rearrange("p (h d) -> p h d", h=HPG), V_CAST)

        def qk_chunks(tb):
            return [lambda tb=tb, pr=pr, w=w: emit_qk_proj(tb, pr, w)
                    for pr in range(2) for w in range(2)]

        def v_chunks(tb):
            return [lambda st=st: emit_v_proj(st)
                    for st in range(4 * tb, 4 * tb + 4)]

        # ------- stage-2 piece (one head PAIR of one t-block) ------
        def emit_headpair(qb, pr, attn_pair):
            """Causal attention s-loop for both heads of pair pr. Scores are
            k-compensated fp8 DoubleRow; one bf16 exp covers both heads;
            diagonal tiles are column-sliced to the causal region. Yields
            once per s-tile so the driver can weave PE filler work in."""
            t0 = qb * TBLK
            ns = 4 * (qb + 1)
            attv = [
                psB.tile([VW, TBLK], F32, tag="attv",
                         name=f"attv{qb}_{pr}_{par}")
                for par in range(2)
            ]
            def emit_av(si):
                ka = si * P - t0 if si * P >= t0 else 0
                ptv = pts[si]
                for par in range(2):
                    h = 2 * pr + par
                    nc.tensor.matmul(
                        attv[par][:, ka:TBLK],
                        lhsT=v_ap()[:, si, h],
                        rhs=ptv[:, par, ka:],
                        start=(si == 0), stop=(si == ns - 1),
                    )

            pts = {}
            for si in range(ns):
                diag = si * P >= t0
                ka = si * P - t0 if diag else 0
                stp = ps2.tile([P, 2 * TBLK], F32, tag="st2",
                               name=f"st{qb}_{pr}_{si}")
                stv = stp[:].rearrange("p (par t) -> p par t", par=2)
                for par in range(2):
                    r0 = par * HS
                    nc.tensor.matmul(
                        stv[:, par, ka:],
                        lhsT=k8_ap(pr)[r0:r0 + HS, si],
                        rhs=q8_ap(pr)[r0:r0 + HS, qb, :, ka:],
                        start=True, stop=True, perf_mode=DR,
                    )
                pt = ptp.tile([P, 2 * TBLK], BF16, tag="pt",
                              name=f"pt{qb}_{pr}_{si}")
                ptv = pt[:].rearrange("p (par t) -> p par t", par=2)
                pts[si] = ptv
                nc.scalar.activation(
                    ptv[:, :, ka:], stv[:, :, ka:], EXP, scale=EXP_SCALE)
                if diag:
                    with nc.allow_low_precision(reason="bf16 causal mask "
                                                "multiply on bf16 P"):
                        nc.vector.tensor_mul(
                            ptv[:, :, ka:ka + P], ptv[:, :, ka:ka + P],
                            tri_sb[:].rearrange("p (two s) -> p two s",
                                                two=2))
                # software pipeline: AV runs one s-tile behind, and PE
                # filler work (injected at the yield) sits between the
                # scores matmul and the AV so it runs during the exp flight
                yield
                if si > 5:
                    emit_av(si - 6)
            for s_ in range(max(0, ns - 6), ns):
                emit_av(s_)
            # normalize: reciprocal of the denominator row (row 64 of attv),
            # Pool-broadcast across partitions, multiply into bf16 att.
            for par in range(2):
                r0 = par * HS
                recip = smp.tile([1, TBLK], F32, tag="recip")
                with nc.allow_low_precision(
                    reason="f32 reciprocal of softmax denominators"
                ):
                    nc.vector.reciprocal(recip[:], attv[par][HS:HS + 1, :])
                bc = smp.tile([HS, TBLK], F32, tag="bcast")
                nc.gpsimd.partition_broadcast(bc[:], recip[:])
                with nc.allow_low_precision(reason="bf16 attention output "
                                            "is the measured-error design"):
                    nc.vector.tensor_mul(
                        attn_pair[r0:r0 + HS, :], attv[par][0:HS, :], bc[:])

        # ---------------- stage-3 piece (one t-tile of one t-block) ---------
        def lh_slice(lh, hdt, c0, c1):
            lht, base = lh[hdt]
            return lht[:, base + c0:base + c1]

        def emit_oproj_tt(qb, lh, tt):
            t0 = qb * TBLK
            op = psC.tile([P, ES], F32, tag="small", name=f"op{qb}_{tt}")
            # pr0 tiles (even hdt) first: they arrive one AllGather earlier
            order = [0, 2, 4, 6, 1, 3, 5, 7]
            for i, hdt in enumerate(order):
                nc.tensor.matmul(
                    op[:],
                    lhsT=lh_slice(lh, hdt, tt * P, (tt + 1) * P),
                    rhs=wo_ap()[:, hdt],
                    start=(i == 0),
                    stop=(i == NCT - 1),
                )
            osb = outp.tile([P, ES], F32, tag="osb", name=f"osb{qb}_{tt}")
            # bias is added during the PSUM->SBUF copy against a
            # pre-broadcast [128, ES] bias tile (no PE bias matmul)
            nc.vector.scalar_tensor_tensor(
                osb[:], op[:], 1.0, bias_bc[:],
                op0=mybir.AluOpType.mult, op1=mybir.AluOpType.add,
            )
            if qb >= 2:
                nc.scalar.dma_start(
                    out[t0 + tt * P:t0 + (tt + 1) * P, :], osb[:])
            else:
                nc.gpsimd.dma_start(
                    out[t0 + tt * P:t0 + (tt + 1) * P, :], osb[:])

        # --------- per-pair AllGather (pr = head pair 0/1 of this core) -----
        # Output rows are rank-major: block g holds GLOBAL heads
        # (4g+2pr, 4g+2pr+1) = wo-row tile index 2g+pr.
        def emit_ag(qb, pr, attn_pair, lh, last=False):
            # For the final AllGather the exps are done, so the otherwise
            # idle ACT DGE queue takes half the chain and the issue
            # overheads run in parallel with the SP queue's.
            q2 = nc.scalar if last else nc.sync
            ag_out = dramp.tile([GROUPS * P, TBLK], BF16,
                                tag=f"agout{qb}_{pr}")
            if with_collective:
                ag_in = dramp.tile([P, TBLK], BF16, tag=f"agin{qb}_{pr}")
                nc.sync.dma_start(ag_in[:], attn_pair[:])
                nc.gpsimd.collective_compute(
                    "AllGather",
                    mybir.AluOpType.bypass,
                    replica_groups=REPLICA_GROUPS,
                    ins=[ag_in[:].opt()],
                    outs=[ag_out[:].opt()],
                )
            else:
                # timing/sim variant: byte-equivalent local DMAs (the input
                # staging write plus one write per gathered block)
                for g_ in range(GROUPS):
                    eng = q2 if g_ % 2 else nc.sync
                    eng.dma_start(
                        ag_out[g_ * P:(g_ + 1) * P, :], attn_pair[:])
            # per-block lh loads: block g only waits its own gather write,
            # so out-proj matmuls start as blocks land instead of waiting
            # for one big load
            for g_ in range(GROUPS):
                lhg = lhp.tile([P, TBLK], BF16, tag="lh",
                               name=f"lh{qb}_{pr}_{g_}")
                q2.dma_start(lhg[:], ag_out[g_ * P:(g_ + 1) * P, :])
                lh[2 * g_ + pr] = (lhg, 0)
            if not with_collective:
                ag_in = dramp.tile([P, TBLK], BF16, tag=f"agin{qb}_{pr}")
                nc.sync.dma_start(ag_in[:], attn_pair[:])

        # ---------------- emission schedule ----------------
        # Upfront loads: weights for QK first, then x t-block by t-block,
        # then V/out-proj weights; constants via memset (no DMA).
        nc.gpsimd.memset(ones[:], 1.0)
        nc.gpsimd.memset(v_ap()[:, :, :, HS:VW], 1.0)

        # PE warmup: dependency-free matmuls on memset data ramp the PE to
        # full clock while the x/weight DMAs are in flight, so the first
        # real projection runs at 2.4 GHz instead of the cold p-state.
        wu = smp.tile([P, TBLK], BF16, tag="bcast", name="warmup_in")
        nc.gpsimd.memset(wu[:], 0.0)
        wups = psC.tile([P, TBLK], F32, tag="small", name="warmup_ps")
        NWU = 8
        for i in range(NWU):
            nc.tensor.matmul(
                wups[:], lhsT=wu[:, 0:P], rhs=wu[:],
                start=(i == 0), stop=(i == NWU - 1),
            )
        wuo = smp.tile([1, TBLK], F32, tag="recip", name="warmup_out")
        nc.vector.tensor_copy(wuo[:], wups[0:1, :])

        def x_dma(tb):
            nc.sync.dma_start(
                x_ap()[:, :, :, tb],
                x8[:].rearrange("(ci p) (hl tb t) -> p hl ci tb t",
                                p=P, hl=2, tb=NTB)[:, :, :, tb],
            )

        def w_dma(t_sb, d):
            nc.sync.dma_start(
                t_sb[:].rearrange("p (ci f) -> p ci f", ci=NCT),
                d[:].rearrange("(ci p) f -> p ci f", p=P),
            )

        # wq + the first x t-block first, so stage-1 starts ASAP; wv before
        # x1 so the V(tb0) chunks don't stall the early-loop PE
        w_dma(wqh_sb, wqh)
        x_dma(0)
        w_dma(wql_sb, wql)
        nc.sync.dma_start(tri_sb[:], tri2[:])
        w_dma(wkh_sb, wkh)
        w_dma(wkl_sb, wkl)
        x_dma(1)
        w_dma(wvh_sb, wvh)
        w_dma(wvl_sb, wvl)
        x_dma(2)
        x_dma(3)
        nc.sync.dma_start(
            wo_sb[:].rearrange("p (ci f) -> p ci f", ci=NCT),
            wo[:].rearrange("(ci p) f -> p ci f", p=P),
        )
        nc.sync.dma_start(bias_sb[:], bo[:])
        nc.gpsimd.partition_broadcast(bias_bc[:], bias_sb[:])

        for chunk in qk_chunks(0) + v_chunks(0):
            chunk()

        def drive_pair(qb, pr, attn_pair, vfill, fillers, stride, off=0):
            """Drive one head pair's s-loop, weaving V fillers (odd units)
            and other fillers (every `stride` units after `off`)."""
            ctr = 0
            for _ in emit_headpair(qb, pr, attn_pair):
                ctr += 1
                if vfill and ctr % 2 == 1:
                    vfill.pop(0)()
                elif (fillers and ctr > off
                      and (ctr - off) % stride == 0):
                    fillers.pop(0)()

        lh_of = {}
        ap_of = {}

        def new_attn_pair(qb):
            ap_of[qb] = [
                attp.tile([P, TBLK], BF16, tag=f"attn{p_}",
                          name=f"at{qb}_{p_}")
                for p_ in range(2)
            ]
            lh_of[qb] = [None] * NCT
            return ap_of[qb]

        def oproj_fillers(qb):
            return [(lambda tt=tt, q=qb: emit_oproj_tt(q, lh_of[q], tt))
                    for tt in range(4)]

        # ---- t-blocks 0 and 1: sequential. Stage-1 chunks (QK/V of later
        # t-blocks) fill these early loops; all out-projections are deferred
        # to the late loops, which have no stage-1 work left.
        ap0, ap1 = new_attn_pair(0), new_attn_pair(1)
        f0 = qk_chunks(1) + v_chunks(1)
        drive_pair(0, 0, ap0[0], [], f0, 1)
        emit_ag(0, 0, ap0[0], lh_of[0])
        drive_pair(0, 1, ap0[1], [], f0, 1)
        while f0:
            f0.pop(0)()
        emit_ag(0, 1, ap0[1], lh_of[0])

        f1 = qk_chunks(2) + v_chunks(2)
        drive_pair(1, 0, ap1[0], [], f1, 1)
        emit_ag(1, 0, ap1[0], lh_of[1])
        drive_pair(1, 1, ap1[1], [], f1, 1)
        while f1:
            f1.pop(0)()
        emit_ag(1, 1, ap1[1], lh_of[1])

        # ---- t-blocks 2 and 3: interleaved at head-pair granularity.
        # Remaining stage-1 work and the deferred out-projections are
        # spread across these ACT-bound loops to keep the PE fed.
        # ---- late pairs: the next pair's first scores are emitted BEFORE
        # the previous pair's AV-drain + normalize, so the ACT exp pipeline
        # never starves across pair boundaries.
        ap2, ap3 = new_attn_pair(2), new_attn_pair(3)
        PRE = 3

        def pump(gen, n):
            for _ in range(n):
                next(gen)

        def run_yields(gen, n, ctr, fillers, stride, off=0):
            for _ in range(n):
                next(gen)
                ctr += 1
                if fillers and ctr > off and (ctr - off) % stride == 0:
                    fillers.pop(0)()

        def finish(gen, fillers=None):
            for _ in gen:
                pass
            while fillers:
                fillers.pop(0)()

        f23 = qk_chunks(3) + v_chunks(3)
        g20 = emit_headpair(2, 0, ap2[0])
        run_yields(g20, 12, 0, f23, 1)
        g30 = emit_headpair(3, 0, ap3[0])
        pump(g30, PRE)
        finish(g20, f23)
        emit_ag(2, 0, ap2[0], lh_of[2])

        of0 = oproj_fillers(0)
        run_yields(g30, 16 - PRE, PRE, of0, 4)
        g21 = emit_headpair(2, 1, ap2[1])
        pump(g21, PRE)
        finish(g30, of0)
        emit_ag(3, 0, ap3[0], lh_of[3])

        of1 = oproj_fillers(1)
        run_yields(g21, 12 - PRE, PRE, of1, 3)
        g31 = emit_headpair(3, 1, ap3[1])
        pump(g31, PRE)
        finish(g21, of1)
        emit_ag(2, 1, ap2[1], lh_of[2])

        # oproj(2) is NOT woven into the (3,1) loop (which is ACT-bound —
        # PE filler is wasted there); it is held back to cover the final
        # AllGather's flight in the tail, where PE would otherwise idle.
        run_yields(g31, 16 - PRE, PRE, [], 99)
        finish(g31)
        of2 = oproj_fillers(2)

        # tail: out-projection of the last t-block; bias + pr0 hd-tiles
        # first so PE has work while the final AllGather is in flight.
        lhz = lh_of[NTB - 1]
        tz = (NTB - 1) * TBLK

        def open_tail_evens(tts, pool=None, tag="st2"):
            pool = pool or ps2
            ops = {}
            for tt in tts:
                # tail groups live in the st2 banks: free after the last
                # exp, so their WAR guard resolves at exp time rather than
                # at the end of the final pair's normalize chain
                op = pool.tile([P, ES], F32, tag=tag, name=f"opz{tt}")
                nc.tensor.matmul(
                    op[:], lhsT=ones[0:1, :], rhs=bias_sb[:],
                    start=True, stop=False,
                )
                for hdt in (0, 2, 4, 6):
                    nc.tensor.matmul(
                        op[:],
                        lhsT=lh_slice(lhz, hdt, tt * P, (tt + 1) * P),
                        rhs=wo_ap()[:, hdt],
                        start=False, stop=False,
                    )
                ops[tt] = op
            return ops

        # the first two t-tiles' even-half groups are emitted BEFORE the
        # final AllGather so their semaphore thresholds (and hence start
        # time) pre-date the gather chain they don't depend on
        ops = open_tail_evens((0, 1))
        for f in of2:
            f()
        ops.update(open_tail_evens((2, 3), psC, "small"))
        emit_ag(3, 1, ap3[1], lh_of[3], last=True)
        for grp in range(2):
            tts = (2 * grp, 2 * grp + 1)
            for tt in tts:
                for j, hdt in enumerate((1, 3, 5, 7)):
                    nc.tensor.matmul(
                        ops[tt][:],
                        lhsT=lh_slice(lhz, hdt, tt * P, (tt + 1) * P),
                        rhs=wo_ap()[:, hdt],
                        start=False, stop=(j == 3),
                    )
                osb = outp.tile([P, ES], F32, tag="osb", name=f"osbz{tt}")
                # endgame: ACT/DVE alternate so the final stores drain in
                # parallel instead of serializing on one engine
                if tt % 2 == 0:
                    nc.scalar.activation(
                        osb[:], ops[tt][:],
                        mybir.ActivationFunctionType.Copy, scale=1.0)
                    nc.scalar.dma_start(
                        out[tz + tt * P:tz + (tt + 1) * P, :], osb[:])
                else:
                    nc.vector.tensor_copy(osb[:], ops[tt][:])
                    nc.sync.dma_start(
                        out[tz + tt * P:tz + (tt + 1) * P, :], osb[:])

    nc.compile()
    return nc


_NC_CACHE = {}


def _get_nc(with_collective=True):
    key = with_collective
    if key not in _NC_CACHE:
        _NC_CACHE[key] = build_nc(with_collective)
    return _NC_CACHE[key]


def _f8(a):
    return a.astype(E4)


def _split8(a, scale):
    hi = _f8(scale * a)
    lo = _f8(scale * a - hi.astype(np.float32))
    return hi, lo


def make_in_maps(x, Wq, Wk, Wv, Wo, bo):
    tri = np.triu(np.ones((P, P), dtype=np.float32))
    tri2 = np.concatenate([tri, tri], axis=1).astype(BF)
    in_maps = []
    for c in range(N_CORES):
        b, g = c // GROUPS, c % GROUPS
        hs_ = slice(g * HPG, (g + 1) * HPG)

        # x8: [C, hl(2), tb(4), t(512)] -> [C, 2T]
        xT = np.ascontiguousarray(x[b].T)            # [C, T]
        x_hi, x_lo = _split8(xT, SX)
        x8 = np.stack([x_hi, x_lo], axis=1)          # [C, 2, T]
        x8 = x8.reshape(C, 2, NTB, TBLK).reshape(C, 2 * T)

        def prep_w(W):
            # W[hs_] -> [C, HD] in (pr, par, hs) column order
            Wl = W[hs_].transpose(1, 0, 2).reshape(C, HD)
            hi, lo = _split8(Wl, SW)
            # hi duplicated per pr block: [C, pr, 2, 128]
            hid = hi.reshape(C, 2, P)
            hid = np.stack([hid, hid], axis=2).reshape(C, 2 * HD)
            return np.ascontiguousarray(hid), np.ascontiguousarray(lo)

        wqh_, wql_ = prep_w(Wq)
        wkh_, wkl_ = prep_w(Wk)
        # V: hi duplicated as one [C, 2, 256] block (no pr split)
        Wvl_ = Wv[hs_].transpose(1, 0, 2).reshape(C, HD)
        v_hi, v_lo = _split8(Wvl_, SW)
        wvh_ = np.ascontiguousarray(
            np.stack([v_hi, v_hi], axis=1).reshape(C, 2 * HD))

        in_maps.append({
            "x8": np.ascontiguousarray(x8),
            "wqh": wqh_, "wkh": wkh_, "wvh": wvh_,
            "wql": wql_, "wkl": wkl_,
            "wvl": np.ascontiguousarray(v_lo),
            "wo": np.ascontiguousarray(Wo[:, g * ES:(g + 1) * ES]).astype(BF),
            "bo": np.ascontiguousarray(
                bo[g * ES:(g + 1) * ES].reshape(1, ES)).astype(BF),
            "tri2": tri2,
        })
    return in_maps


def kernel(x, Wq, Wk, Wv, Wo, bo):
    x = np.asarray(x, dtype=np.float32)
    Wq = np.asarray(Wq, dtype=np.float32)
    Wk = np.asarray(Wk, dtype=np.float32)
    Wv = np.asarray(Wv, dtype=np.float32)
    Wo = np.asarray(Wo, dtype=np.float32)
    bo = np.asarray(bo, dtype=np.float32)

    nc = _get_nc(with_collective=True)
    in_maps = make_in_maps(x, Wq, Wk, Wv, Wo, bo)
    res = run_bass_kernel_spmd(nc, in_maps, core_ids=list(range(N_CORES)))

    out = np.empty((B, T, E), dtype=np.float32)
    for c in range(N_CORES):
        b, g = c // GROUPS, c % GROUPS
        out[b, :, g * ES:(g + 1) * ES] = res.results[c]["out"]
    return out
